# revision 34
# baseline (speedup 1.0000x reference)
"""Trainium2 Bass kernel for the LSTM-modulated linear recurrence module.

Reference semantics (per batch element b, scalar input x_t):
    z_t = W_ih x_t + b_ih + b_hh + W_hh h_{t-1}          (8 gate pre-acts, HID=2)
    c_t = sigmoid(f_t) c_{t-1} + sigmoid(i_t) tanh(g_t)
    h_t = sigmoid(o_t) tanh(c_t)
    y_t = h1_t * y_{t-1} + x_t * h0_t                    (linear scan, y_0 = 0)

Strategy:
  * Pure data parallel over batch: 4096 -> 512 per core across 8 cores.
  * Per core the T=4096 recurrence is split into NCH chunks of C steps plus
    a K-step halo; the LSTM state contracts ~0.77x/step, so a zero-state
    start K steps early reconverges to the true state below fp32 noise
    (chunk 0 is reset to the exact zero initial state at step K).
    All chunks advance in lockstep: effective batch 512*NCH per step.
  * Gate pre-activations in bf16 on VectorE (2x mode); activations and the
    c/h state in fp32.  The x-dependent part of z is precomputed in slabs
    with tensor_scalar (weights baked as immediates at build time).
  * h history is spilled to HBM fp32 via a ring buffer; phase 2 computes
    y with the DVE tensor_tensor_scan instruction.
"""

import numpy as np
import ml_dtypes
from contextlib import ExitStack

import concourse.bass as bass
import concourse.tile as tile
from concourse import bacc, mybir
from concourse.bass_utils import run_bass_kernel_spmd

AF = mybir.ActivationFunctionType
ALU = mybir.AluOpType
F32 = mybir.dt.float32
BF16 = mybir.dt.bfloat16

P = 128
N_CORES = 8
B_TOT = 4096
T_TOT = 4096
B_LOC = B_TOT // N_CORES          # 512
BT = B_LOC // P                   # 4 batch tiles
import os
C = int(os.environ.get("KERN_C", "128"))  # chunk length
K = int(os.environ.get("KERN_K", "32"))  # halo (warmup) steps
NCH = T_TOT // C                  # chunks
STEPS = C + K                     # sequential steps
G = BT * NCH                      # lanes per partition per gate-column
SSLAB = 256 // NCH                # xp precompute slab (steps)
SB = 512 // NCH                   # hist ring slab (steps); ring = 16KB/partition

# gate order in the z layout: [f0 f1 i0 i1 g0 g1 o0 o1]
JORDER = [2, 3, 0, 1, 4, 5, 6, 7]

_CACHE = {}


def _build_program(W_ih, W_hh, b_ih, b_hh):
    w0 = np.asarray(W_hh, np.float64)[JORDER, 0]
    w1 = np.asarray(W_hh, np.float64)[JORDER, 1]
    wx = np.asarray(W_ih, np.float64)[JORDER, 0]
    uu = (np.asarray(b_ih, np.float64) + np.asarray(b_hh, np.float64))[JORDER]
    # tanh(g) = 2*sigmoid(2g) - 1: double the g-gate (jc 6,7) weights so one
    # Sigmoid instruction covers all 8 gate columns.
    for jj in (4, 5):
        w0[jj] *= 2.0; w1[jj] *= 2.0; wx[jj] *= 2.0; uu[jj] *= 2.0

    nc = bacc.Bacc("TRN2", target_bir_lowering=False, debug=False)

    # x in step-major unfolded layout
    x_d = nc.dram_tensor("x_ts", [P, STEPS, BT, NCH], F32, kind="ExternalInput")
    y_d = nc.dram_tensor("y", [B_LOC, T_TOT], F32, kind="ExternalOutput")
    hist0_d = nc.dram_tensor("hist0_scr", [P, BT, NCH, C], F32)
    hist1_d = nc.dram_tensor("hist1_scr", [P, BT, NCH, C], BF16)

    with tile.TileContext(nc) as tc:
        with ExitStack() as ctx:
            cpool = ctx.enter_context(tc.tile_pool(name="consts", bufs=1))
            x_sb = cpool.tile([P, STEPS, BT, NCH], F32, tag="x")
            wt01 = cpool.tile([P, 8, 2, G], BF16, tag="wt01")
            mask0 = cpool.tile([P, 2, BT, NCH], BF16, tag="mask0")

            nc.sync.dma_start(x_sb[:, 0:16], x_d[:, 0:16])
            nc.sync.dma_start(x_sb[:, 16:64], x_d[:, 16:64])
            nc.sync.dma_start(x_sb[:, 64:STEPS], x_d[:, 64:STEPS])
            for j in range(8):
                nc.vector.memset(wt01[:, j, 0, :], float(w0[j]))
                nc.vector.memset(wt01[:, j, 1, :], float(w1[j]))
            nc.vector.memset(mask0[:], 1.0)
            for comp in range(2):
                for bt in range(BT):
                    nc.vector.memset(mask0[:, comp, bt, 0:1], 0.0)

            with ExitStack() as p1:
                hbpool = p1.enter_context(tc.tile_pool(name="hbp", bufs=8))
                cpool2 = p1.enter_context(tc.tile_pool(name="cp", bufs=8))
                mpool = p1.enter_context(tc.tile_pool(name="mp", bufs=4))
                spool = p1.enter_context(tc.tile_pool(name="sp", bufs=6))
                xppool = p1.enter_context(tc.tile_pool(name="xpp", bufs=4))
                rpool = p1.enter_context(tc.tile_pool(name="rp", bufs=2))

                NCO = 2                   # cohorts (independent chunk groups)
                GC = G // NCO             # lanes per cohort
                CHC = NCH // NCO          # chunks per cohort

                def gen_xp(co, s0):
                    ns = min(SSLAB, STEPS - s0)
                    t = xppool.tile([P, SSLAB, 8, GC], BF16, tag="xp")
                    for j in range(8):
                        o_ap = t[:, 0:ns, j, :].rearrange(
                            "p s (b c) -> p s b c", b=BT
                        )
                        i_ap = x_sb[:, s0 : s0 + ns, :,
                                    co * CHC : (co + 1) * CHC]
                        nc.scalar.activation(
                            o_ap, i_ap, AF.Copy,
                            bias=float(uu[j]), scale=float(wx[j]),
                        )
                    return t

                st = []
                for co in range(NCO):
                    h_bf = hbpool.tile([P, 2 * GC], BF16, tag="hb")
                    c_cur = cpool2.tile([P, 2 * GC], F32, tag="c")
                    nc.vector.memset(h_bf[:], 0.0)
                    nc.vector.memset(c_cur[:], 0.0)
                    st.append({"h": h_bf, "c": c_cur, "xp": gen_xp(co, 0),
                               "xpn": None})
                ring = None

                for s in range(STEPS):
                    if (s - K) % SB == 0 and s >= K:
                        ring = rpool.tile([P, BT, NCH, SB], F32, tag="ring0")
                        ring1 = rpool.tile([P, BT, NCH, SB], BF16, tag="ring1")
                    for co in range(NCO):
                        S = st[co]
                        if s % SSLAB == SSLAB // 2 and s + SSLAB // 2 < STEPS:
                            S["xpn"] = gen_xp(co, s + SSLAB - SSLAB // 2)
                        if s % SSLAB == 0 and s > 0:
                            S["xp"] = S["xpn"]
                        if s == K and co == 0:
                            # chunk 0 warmup used zero-padded x; its true
                            # initial state is exactly zero.
                            h_m = hbpool.tile([P, 2 * GC], BF16, tag="hb")
                            c_m = cpool2.tile([P, 2 * GC], F32, tag="c")
                            mk = mask0[:, :, :, 0:CHC]
                            v4 = lambda ap: ap.rearrange(
                                "p (a b c) -> p a b c", a=2, b=BT
                            )
                            nc.vector.tensor_mul(v4(h_m[:]), v4(S["h"][:]), mk)
                            nc.vector.tensor_mul(v4(c_m[:]), v4(S["c"][:]), mk)
                            S["h"], S["c"] = h_m, c_m

                        # z = (W0*h0 + W1*h1) + xp
                        hb4 = (
                            S["h"][:]
                            .rearrange("p (c g) -> p c g", c=2)
                            .unsqueeze(1)
                            .broadcast_to((P, 8, 2, GC))
                        )
                        m01 = mpool.tile([P, 8, 2, GC], BF16, tag="m01")
                        nc.vector.tensor_mul(
                            m01[:], hb4, wt01[:, :, :, co * GC : (co + 1) * GC]
                        )
                        a1 = mpool.tile([P, 8, GC], BF16, tag="a1")
                        nc.vector.tensor_add(
                            a1[:], m01[:, :, 0, :], m01[:, :, 1, :]
                        )
                        z = mpool.tile([P, 8, GC], BF16, tag="z")
                        nc.vector.tensor_add(
                            z[:], a1[:], S["xp"][:, s % SSLAB, :, :]
                        )

                        sig = spool.tile([P, 8 * GC], F32, tag="sig")
                        nc.scalar.activation(
                            sig[:, 0 : 6 * GC].rearrange("p (a b) -> p a b", a=6),
                            z[:, 0:6, :], AF.Sigmoid,
                        )
                        nc.scalar.activation(
                            sig[:, 6 * GC : 8 * GC].rearrange(
                                "p (a b) -> p a b", a=2
                            ),
                            z[:, 6:8, :], AF.Sigmoid,
                        )
                        tg = spool.tile([P, 2 * GC], F32, tag="tg")
                        nc.vector.tensor_scalar(
                            out=tg[:], in0=sig[:, 4 * GC : 6 * GC],
                            scalar1=2.0, scalar2=-1.0,
                            op0=ALU.mult, op1=ALU.add,
                        )

                        cA = spool.tile([P, 2 * GC], F32, tag="cA")
                        cB = spool.tile([P, 2 * GC], F32, tag="cB")
                        c_new = cpool2.tile([P, 2 * GC], F32, tag="c")
                        nc.gpsimd.tensor_mul(
                            cA[:], sig[:, 0 : 2 * GC], S["c"][:]
                        )
                        nc.vector.tensor_mul(
                            cB[:], sig[:, 2 * GC : 4 * GC], tg[:]
                        )
                        nc.vector.tensor_add(c_new[:], cA[:], cB[:])

                        tc_t = spool.tile([P, 2 * GC], F32, tag="tc")
                        nc.scalar.activation(tc_t[:], c_new[:], AF.Tanh)
                        h_bf2 = hbpool.tile([P, 2 * GC], BF16, tag="hb")
                        nc.vector.tensor_mul(
                            h_bf2[:], sig[:, 6 * GC : 8 * GC], tc_t[:]
                        )

                        if s >= K:
                            # h history into the HBM-spill rings: h0 fp32
                            # (additive b input), h1 bf16 (decay gate a)
                            nc.gpsimd.tensor_mul(
                                ring[:, :, co * CHC : (co + 1) * CHC,
                                     (s - K) % SB],
                                sig[:, 6 * GC : 7 * GC].rearrange(
                                    "p (b c) -> p b c", b=BT
                                ),
                                tc_t[:, 0:GC].rearrange(
                                    "p (b c) -> p b c", b=BT
                                ),
                            )
                            nc.gpsimd.tensor_mul(
                                ring1[:, :, co * CHC : (co + 1) * CHC,
                                      (s - K) % SB],
                                sig[:, 7 * GC : 8 * GC].rearrange(
                                    "p (b c) -> p b c", b=BT
                                ),
                                tc_t[:, GC : 2 * GC].rearrange(
                                    "p (b c) -> p b c", b=BT
                                ),
                            )
                        S["h"], S["c"] = h_bf2, c_new
                    if s >= K and (s - K) % SB == SB - 1:
                        s0 = (s - K) - (SB - 1)
                        nc.sync.dma_start(
                            hist0_d[:, :, :, s0 : s0 + SB], ring[:]
                        )
                        nc.sync.dma_start(
                            hist1_d[:, :, :, s0 : s0 + SB], ring1[:]
                        )

            # phase 2: y_t = a_t y_{t-1} + b_t via tensor_tensor_scan
            HCH = NCH // 2  # half the chunks = 2048 time steps at once
            with ExitStack() as p2:
                apool = p2.enter_context(tc.tile_pool(name="ap", bufs=4))
                hpool2 = p2.enter_context(tc.tile_pool(name="h0p", bufs=4))
                bpool = p2.enter_context(tc.tile_pool(name="bp", bufs=3))
                ypool = p2.enter_context(tc.tile_pool(name="yp", bufs=3))
                for bt in range(BT):
                    y_prev = None
                    for hf in range(2):
                        ch0 = hf * HCH
                        a_t = apool.tile([P, HCH * C], BF16, tag="a")
                        h0_t = hpool2.tile([P, HCH * C], F32, tag="h0")
                        nc.scalar.dma_start(
                            a_t[:].rearrange("p (c s) -> p c s", c=HCH),
                            hist1_d[:, bt, ch0 : ch0 + HCH, :],
                        )
                        nc.gpsimd.dma_start(
                            h0_t[:].rearrange("p (c s) -> p c s", c=HCH),
                            hist0_d[:, bt, ch0 : ch0 + HCH, :],
                        )
                        # x slice in (ch, s) order from the step-major layout
                        xsl = x_sb[:, K:, bt, ch0 : ch0 + HCH].transpose([0, 2, 1])
                        eng = nc.vector
                        b_t = bpool.tile([P, HCH * C], F32, tag="b")
                        eng.tensor_mul(
                            b_t[:].rearrange("p (c s) -> p c s", c=HCH),
                            xsl,
                            h0_t[:].rearrange("p (c s) -> p c s", c=HCH),
                        )
                        y_t = ypool.tile([P, HCH * C], F32, tag="y")
                        nseg = (HCH * C) // 1024
                        for sg_i in range(nseg):
                            lo = sg_i * 1024
                            init = (
                                0.0
                                if (hf == 0 and sg_i == 0)
                                else (
                                    y_prev[:, HCH * C - 1 : HCH * C]
                                    if sg_i == 0
                                    else y_t[:, lo - 1 : lo]
                                )
                            )
                            eng.tensor_tensor_scan(
                                y_t[:, lo : lo + 1024],
                                a_t[:, lo : lo + 1024],
                                b_t[:, lo : lo + 1024],
                                init,
                                ALU.mult,
                                ALU.add,
                            )
                        nc.sync.dma_start(
                            y_d[bt * P : (bt + 1) * P,
                                hf * HCH * C : (hf + 1) * HCH * C],
                            y_t[:],
                        )
                        y_prev = y_t

    nc.compile()
    return nc


def _host_prep(x):
    """Per-core input maps: x in step-major unfolded layout."""
    xs = np.ascontiguousarray(x[:, :, 0], dtype=np.float32)  # [B, T]
    idx = (np.arange(NCH) * C)[None, :] + np.arange(STEPS)[:, None]  # [STEPS, NCH]
    in_maps = []
    for core in range(N_CORES):
        xc = xs[core * B_LOC : (core + 1) * B_LOC]              # [512, T]
        xp_ = np.concatenate([np.zeros((B_LOC, K), np.float32), xc], axis=1)
        unf = xp_[:, idx]                                       # [512, STEPS, NCH]
        unf = np.ascontiguousarray(
            unf.reshape(BT, P, STEPS, NCH).transpose(1, 2, 0, 3)
        )                                                       # [128, STEPS, BT, NCH]
        in_maps.append({"x_ts": unf})
    return in_maps


def _get_program(W_ih, W_hh, b_ih, b_hh):
    key = (
        np.asarray(W_ih).tobytes(), np.asarray(W_hh).tobytes(),
        np.asarray(b_ih).tobytes(), np.asarray(b_hh).tobytes(),
    )
    if _CACHE.get("key") != key:
        _CACHE["nc"] = _build_program(W_ih, W_hh, b_ih, b_hh)
        _CACHE["key"] = key
    return _CACHE["nc"]


def kernel(x, W_ih, W_hh, b_ih, b_hh):
    nc = _get_program(W_ih, W_hh, b_ih, b_hh)
    in_maps = _host_prep(np.asarray(x))
    res = run_bass_kernel_spmd(nc, in_maps, core_ids=list(range(N_CORES)))
    y = np.concatenate([res.results[c]["y"] for c in range(N_CORES)], axis=0)
    return y[..., None].astype(np.float32)
